# revision 1
# baseline (speedup 1.0000x reference)
"""Trainium2 Bass kernel for nn_BertSelfAttention_79577154060613.

Block-sparse BERT self-attention (block-diagonal over 10 candidate blocks of
64 tokens + dense global columns for 128 term tokens), data-parallel over
batch across 8 NeuronCores (2 batches per core).

Key algorithmic trick: the reference multiplies scores by the mask (masked
entries become exactly 0, not -inf), so softmax gives each masked key weight
exp(0)=1. For a query in block c:
    ctx = (sum_{k in block c | terms} e^{s_k} v_k + sum_{c' != c} Vsum_c') / Z
    Z   = sum_{k in block c | terms} e^{s_k} + 9*64
where Vsum_c' are per-head, per-block sums of candidate value rows. This
turns 768-wide attention into 192-wide attention plus one small K=10 matmul
(lhsT = 1 - one_hot(c)) per query tile.

All tensor-engine inputs are bf16 (fp32 matmuls stream at 1/4 rate on TRN2);
accumulation stays fp32 in PSUM and the softmax divide runs in fp32, so the
end-to-end error stays at the bf16-rounding level (~2e-3 relative).

Layouts (no on-chip transposes anywhere):
  - X^T [h, t]: host pre-transposes and pre-casts hidden_states.
  - Q^T, K^T [o, t] = matmul(lhsT=W^T tile, rhs=X^T); head h lives at
    partitions (h%2)*64 of tile h//2. Biases are added by the PSUM->SBUF
    copy (per-partition tensor_scalar add).
  - V [t, o] = matmul(lhsT=X^T tile, rhs=W^T), stored per head with a ones
    column ([t, 12*(64+1)] bf16) so every PV matmul also accumulates the
    softmax denominator into a 65th PSUM column. V's bias (free-dim) is
    added via a materialized [128, H] bias tile on the copy.
  - scores^T [k, q] = matmul(lhsT=K^T head, rhs=Q^T head); exp on ACT
    (scale=1/8) casting to bf16; the exp'ed scores are the *stationary*
    operand of PV, giving ctx in natural [q, dh] layout, so the divided
    output DMAs straight out.

PSUM discipline: start=True lazily zeroes the whole 2KB bank for the written
partitions, so each bank hosts exactly one accumulation group per partition
half, opened by the correction matmuls (which cover all 4 heads' columns).
"""

import numpy as np
import ml_dtypes

import concourse.bass as bass
import concourse.mybir as mybir
import concourse.tile as tile
from concourse import bacc
from concourse.bass_utils import run_bass_kernel_spmd

# Problem dims (hardcoded per contract)
B, CDD, L, T, H, NH = 16, 10, 64, 128, 768, 12
DH = H // NH  # 64
S = CDD * L + T  # 768
NQ = CDD * L  # 640
P = 128
NCORES = 8
BL = B // NCORES  # 2 batches per core
KT = H // P  # 6 contraction tiles
FP32 = mybir.dt.float32
BF16 = mybir.dt.bfloat16
AF = mybir.ActivationFunctionType
ALU = mybir.AluOpType
HGS = 4  # heads per attention group
NHG = NH // HGS  # 3 groups
VW = DH + 1  # value width per head incl. ones column (65)


def _build_program():
    nc = bacc.Bacc(
        "TRN2", target_bir_lowering=False, debug=False, num_devices=NCORES
    )
    x = nc.dram_tensor("x", [BL, H, S], BF16, kind="ExternalInput").ap()
    wqt = nc.dram_tensor("wqt", [H, H], BF16, kind="ExternalInput").ap()
    wkt = nc.dram_tensor("wkt", [H, H], BF16, kind="ExternalInput").ap()
    wvt = nc.dram_tensor("wvt", [H, H], BF16, kind="ExternalInput").ap()
    bq = nc.dram_tensor("bq", [H], FP32, kind="ExternalInput").ap()
    bk = nc.dram_tensor("bk", [H], FP32, kind="ExternalInput").ap()
    bv16 = nc.dram_tensor("bv16", [H], BF16, kind="ExternalInput").ap()
    out = nc.dram_tensor("out", [BL, S, H], FP32, kind="ExternalOutput").ap()

    with tile.TileContext(nc) as tc:
        _emit(tc, nc, x, wqt, wkt, wvt, bq, bk, bv16, out)
    nc.compile()
    return nc


def _emit(tc, nc, x, wqt, wkt, wvt, bq, bk, bv16, out):
    from contextlib import ExitStack

    ctx = ExitStack()
    with ctx:
        cpool = ctx.enter_context(tc.tile_pool(name="consts", bufs=1))
        wpool = ctx.enter_context(tc.tile_pool(name="weights", bufs=1))
        xtp = ctx.enter_context(tc.tile_pool(name="xt", bufs=2))
        qkv = ctx.enter_context(tc.tile_pool(name="qkv", bufs=2))
        sep = ctx.enter_context(tc.tile_pool(name="se", bufs=3))
        osp = ctx.enter_context(tc.tile_pool(name="osb", bufs=1))
        smp = ctx.enter_context(tc.tile_pool(name="small", bufs=2))
        psp = ctx.enter_context(tc.tile_pool(name="psum", bufs=1, space="PSUM"))

        # ---- constants ----
        onesrow = cpool.tile([1, P], BF16)  # 1.0 row (rank-1 lhsT)
        nc.gpsimd.memset(onesrow[:], 1.0)
        zrow = cpool.tile([1, 1], BF16)  # 0.0 (group-closer rank-1 rhs)
        nc.gpsimd.memset(zrow[:], 0.0)
        # notselC[p, c*64+j] = 0 if p == c else 1  (p in 0..9)
        notselC = cpool.tile([CDD, NQ], BF16)
        nc.gpsimd.memset(notselC[:], 1.0)
        nc.gpsimd.affine_select(
            out=notselC.rearrange("p (c j) -> p c j", j=L),
            in_=notselC.rearrange("p (c j) -> p c j", j=L),
            compare_op=ALU.not_equal,
            fill=0.0,
            base=0,
            pattern=[[-1, CDD], [0, L]],
            channel_multiplier=1,
        )
        # block-membership indicator for Vsums: G[p, j] = 1 iff j-10 == p//64
        G = cpool.tile([P, 20], BF16)
        nc.gpsimd.memset(G[:], 0.0)
        nc.gpsimd.memset(G[0:64, 10:11], 1.0)
        nc.gpsimd.memset(G[64:128, 11:12], 1.0)

        # ---- weights & biases (shared by both batches) ----
        # xt(b=0) + wq chunks are interleaved so the first projection's
        # K-accumulation can start as soon as chunk 0 lands; wk/wv follow.
        w_sb = {}
        w_aps = {"q": wqt, "k": wkt, "v": wvt}
        for name in ("q", "k", "v"):
            w_sb[name] = wpool.tile(
                [P, KT, H], BF16, tag=f"w{name}", name=f"w{name}"
            )
        bvb = cpool.tile([P, H], FP32)  # built right before V projection
        b_col = {}
        bv_row = cpool.tile([1, H], BF16)
        xt0 = []
        for kt in range(KT):
            t = xtp.tile([P, S], BF16, tag=f"xt{kt}", name=f"xt{kt}")
            nc.sync.dma_start(out=t[:], in_=x[0][kt * P : (kt + 1) * P, :])
            nc.sync.dma_start(
                out=w_sb["q"][:, kt, :],
                in_=w_aps["q"].rearrange("(kt p) o -> p kt o", p=P)[:, kt, :],
            )
            xt0.append(t)
            if kt == 0:
                # tiny bias DMAs (needed by the first projection copies)
                for name, bap in (("q", bq), ("k", bk)):
                    bc = cpool.tile([P, KT], FP32, tag=f"bc{name}", name=f"bcol{name}")
                    nc.sync.dma_start(
                        out=bc[:], in_=bap.rearrange("(t p) -> p t", p=P)
                    )
                    b_col[name] = bc
                nc.sync.dma_start(out=bv_row[:], in_=bv16[None, :])
        for name in ("k", "v"):
            wr = w_aps[name].rearrange("(kt p) o -> p kt o", p=P)
            for kt in range(KT):
                nc.sync.dma_start(out=w_sb[name][:, kt, :], in_=wr[:, kt, :])

        xt_next = xt0
        for b in range(BL):
            # ---- X^T (host pre-transposed, bf16; prefetched) ----
            xt = xt_next

            # ---- projections ----
            qt_sb = [qkv.tile([P, NQ], BF16, tag=f"qt{m}", name=f"qt{m}") for m in range(KT)]
            kt_sb = [qkv.tile([P, S], BF16, tag=f"kt{m}", name=f"kt{m}") for m in range(KT)]
            vext = [qkv.tile([P, NH * VW], BF16, tag=f"v{m}", name=f"v{m}") for m in range(KT)]
            vterm = qkv.tile([P, H], FP32, tag="vterm", name="vterm")

            # Q^T, K^T: out[o-tile, t-chunk]; bias added on the copy
            for name, dst, nlen_total in (("q", qt_sb, NQ), ("k", kt_sb, S)):
                for mt in range(KT):
                    ms = slice(mt * P, (mt + 1) * P)
                    n0 = 0
                    while n0 < nlen_total:
                        nlen = min(512, nlen_total - n0)
                        ps = psp.tile([P, 512], FP32, tag="psA", bufs=4, name="psA")
                        for kt in range(KT):
                            nc.tensor.matmul(
                                ps[:, :nlen],
                                lhsT=w_sb[name][:, kt, ms],
                                rhs=xt[kt][:, n0 : n0 + nlen],
                                start=(kt == 0),
                                stop=(kt == KT - 1),
                            )
                        if name == "q":
                            nc.scalar.activation(
                                dst[mt][:, n0 : n0 + nlen],
                                ps[:, :nlen],
                                AF.Identity,
                                bias=b_col[name][:, mt : mt + 1],
                            )
                        else:
                            nc.vector.tensor_scalar_add(
                                dst[mt][:, n0 : n0 + nlen],
                                ps[:, :nlen],
                                b_col[name][:, mt : mt + 1],
                            )
                        n0 += nlen

            if b == 0:
                # materialized V bias [128, H] fp32 (free-dim bias add on copy)
                for n0, nlen in ((0, 512), (512, 256)):
                    ps = psp.tile([P, 512], FP32, tag="psA", bufs=4, name="psA")
                    nc.tensor.matmul(
                        ps[:, :nlen],
                        lhsT=onesrow[:],
                        rhs=bv_row[0:1, n0 : n0 + nlen],
                        start=True,
                        stop=True,
                    )
                    nc.vector.tensor_copy(bvb[:, n0 : n0 + nlen], ps[:, :nlen])

            # V: out[t-tile, o-chunk] -> vext (bf16, 65-strided) + vterm fp32
            for mt in range(KT):
                ms = slice(mt * P, (mt + 1) * P)
                for n0, nlen in ((0, 512), (512, 256)):
                    ps = psp.tile([P, 512], FP32, tag="psA", bufs=4, name="psA")
                    for kt in range(KT):
                        nc.tensor.matmul(
                            ps[:, :nlen],
                            lhsT=xt[kt][:, ms],
                            rhs=w_sb["v"][:, kt, n0 : n0 + nlen],
                            start=(kt == 0),
                            stop=(kt == KT - 1),
                        )
                    nh0 = n0 // DH
                    nheads = nlen // DH
                    vv = vext[mt].rearrange("p (h c) -> p h c", c=VW)
                    nc.vector.tensor_tensor(
                        out=vv[:, nh0 : nh0 + nheads, 0:DH],
                        in0=ps[:, :nlen].rearrange("p (h c) -> p h c", c=DH),
                        in1=bvb[:, n0 : n0 + nlen].rearrange("p (h c) -> p h c", c=DH),
                        op=ALU.add,
                    )
                    if mt == KT - 1:
                        # fp32 copy of term-value rows for output passthrough
                        nc.scalar.activation(
                            vterm[:, n0 : n0 + nlen],
                            ps[:, :nlen],
                            AF.Copy,
                        )
                vv = vext[mt].rearrange("p (h c) -> p h c", c=VW)
                nc.gpsimd.memset(vv[:, :, DH : DH + 1], 1.0)
            # vterm still needs the bias
            nc.vector.tensor_add(vterm[:], vterm[:], bvb[:])
            # term rows pass through V (fp32, bias included) - DMA out early
            nc.sync.dma_start(out=out[b][NQ:S, :], in_=vterm[:])

            # ---- per-block value sums, stored with 65th col = 64.0 so the
            # notselC correction matmul also contributes 9*64=576 to Z ----
            vsumsE = smp.tile([CDD, NH * VW], BF16, tag="vsums", name="vsumsE")
            for n0 in (0, 384):
                ps = psp.tile([P, 512], FP32, tag="psA", bufs=4, name="psA")
                nh0 = n0 // DH
                for kt in range(5):
                    rhs = vext[kt].rearrange("p (h c) -> p h c", c=VW)[
                        :, nh0 : nh0 + 6, 0:DH
                    ]
                    nc.tensor.matmul(
                        ps[0:CDD, 0:384],
                        lhsT=G[:, 10 - 2 * kt : 20 - 2 * kt],
                        rhs=rhs,
                        start=(kt == 0),
                        stop=(kt == 4),
                    )
                vsv = vsumsE.rearrange("p (h c) -> p h c", c=VW)
                nc.vector.tensor_copy(
                    vsv[:, nh0 : nh0 + 6, 0:DH],
                    ps[0:CDD, 0:384].rearrange("p (h c) -> p h c", c=DH),
                )
            vsv = vsumsE.rearrange("p (h c) -> p h c", c=VW)
            nc.gpsimd.memset(vsv[:, :, DH : DH + 1], float(L))

            # prefetch next batch's X^T while attention runs (SWDGE path)
            if b + 1 < BL:
                xt_next = []
                for kt in range(KT):
                    t = xtp.tile([P, S], BF16, tag=f"xt{kt}", name=f"xt{kt}")
                    nc.sync.dma_start(
                        out=t[:], in_=x[b + 1][kt * P : (kt + 1) * P, :]
                    )
                    xt_next.append(t)

            # ---- attention ----
            def emit_scores(hg):
                se_t = [sep.tile([P, NQ], BF16, tag=f"set{i}", name=f"set{i}") for i in range(HGS)]
                se_b = [sep.tile([P, 5 * L], BF16, tag=f"seb{i}", name=f"seb{i}") for i in range(HGS)]
                for hl in range(HGS):
                    hh = hg * HGS + hl
                    pt, r0 = hh // 2, (hh % 2) * 64
                    QTh = qt_sb[pt][r0 : r0 + 64, :]
                    KTh = kt_sb[pt][r0 : r0 + 64, :]
                    # term scores^T [128 terms, 640 q]
                    for n0 in (0, 320):
                        ps = psp.tile([P, 512], FP32, tag="psA", bufs=4, name="psA")
                        nc.tensor.matmul(
                            ps[:, 0:320],
                            lhsT=KTh[:, NQ:S],
                            rhs=QTh[:, n0 : n0 + 320],
                            start=True,
                            stop=True,
                        )
                        nc.scalar.activation(
                            se_t[hl][:, n0 : n0 + 320],
                            ps[:, 0:320],
                            AF.Exp,
                            scale=0.125,
                        )
                    # block scores^T: all 10 blocks in one psum bank
                    ps = psp.tile([P, 5 * L], FP32, tag="psB", bufs=1, name="psB", padded_shape=[P, 512])
                    for j in range(5):
                        for half in (0, 1):
                            c = 2 * j + half
                            cs = slice(c * L, (c + 1) * L)
                            nc.tensor.matmul(
                                ps[half * 64 : half * 64 + 64, j * L : (j + 1) * L],
                                lhsT=KTh[:, cs],
                                rhs=QTh[:, cs],
                                start=True,
                                stop=True,
                            )
                    nc.scalar.activation(
                        se_b[hl][:],
                        ps[:],
                        AF.Exp,
                        scale=0.125,
                    )
                return se_t, se_b

            def emit_pv(hg, se_t, se_b):
                for j in range(5):
                    psc = psp.tile([P, HGS * VW], FP32, tag="psC", bufs=3, name="psC", padded_shape=[P, 512])
                    hgs_v = slice(hg * HGS * VW, (hg + 1) * HGS * VW)
                    # head 0's full-height terms matmul opens the bank's one
                    # accumulation group; everything else accumulates.
                    for hl in range(HGS):
                        hh = hg * HGS + hl
                        vs = slice(hh * VW, (hh + 1) * VW)
                        nc.tensor.matmul(
                            psc[:, hl * VW : (hl + 1) * VW],
                            lhsT=se_t[hl][:, j * P : (j + 1) * P],
                            rhs=vext[5][:, vs],
                            start=(hl == 0),
                            stop=False,
                        )
                    for half in (0, 1):
                        c = 2 * j + half
                        hs = slice(half * 64, half * 64 + 64)
                        nc.tensor.matmul(
                            psc[hs, :],
                            lhsT=notselC[:, c * L : (c + 1) * L],
                            rhs=vsumsE[:, hgs_v],
                            start=False,
                            stop=False,
                        )
                    for hl in range(HGS):
                        hh = hg * HGS + hl
                        c0 = hl * VW
                        vs = slice(hh * VW, (hh + 1) * VW)
                        for half in (0, 1):
                            hs = slice(half * 64, half * 64 + 64)
                            nc.tensor.matmul(
                                psc[hs, c0 : c0 + VW],
                                lhsT=se_b[hl][hs, j * L : (j + 1) * L],
                                rhs=vext[j][hs, vs],
                                start=False,
                                stop=False,
                            )
                    # full-height +0 rank-1 whose stop closes the bank's group
                    nc.tensor.matmul(
                        psc[:, DH : DH + 1],
                        lhsT=onesrow[:],
                        rhs=zrow[:],
                        start=False,
                        stop=True,
                    )
                    zr = smp.tile([P, HGS], FP32, tag="zr", bufs=4, name="zr")
                    pscv = psc.rearrange("p (h c) -> p h c", c=VW)
                    nc.vector.reciprocal(
                        zr[:].rearrange("p (h o) -> p h o", o=1),
                        pscv[:, :, DH : DH + 1],
                    )
                    ob = osp.tile([P, HGS * DH], FP32, tag=f"osb{j}", bufs=2, name=f"osb{j}")
                    in0 = pscv[:, :, 0:DH]
                    in1 = zr[:].rearrange("p (h o) -> p h o", o=1)
                    bin0, bin1 = bass.broadcast_tensor_aps(in0, in1)
                    nc.vector.tensor_tensor(
                        out=ob[:].rearrange("p (h c) -> p h c", c=DH),
                        in0=bin0,
                        in1=bin1,
                        op=ALU.mult,
                    )
                    nc.sync.dma_start(
                        out=out[b][j * P : (j + 1) * P, hg * HGS * DH : (hg + 1) * HGS * DH],
                        in_=ob[:],
                    )

            prev = None
            for hg in range(NHG):
                cur = emit_scores(hg)
                if prev is not None:
                    emit_pv(hg - 1, *prev)
                prev = cur
            emit_pv(NHG - 1, *prev)

_CACHE = {}


def _get_program():
    if "nc" not in _CACHE:
        _CACHE["nc"] = _build_program()
    return _CACHE["nc"]


def _make_in_maps(inputs):
    hs = np.asarray(inputs["hidden_states"], np.float32)
    hst = np.ascontiguousarray(hs.transpose(0, 2, 1)).astype(ml_dtypes.bfloat16)
    wq = np.asarray(inputs["Wq"], np.float32)
    wk = np.asarray(inputs["Wk"], np.float32)
    wv = np.asarray(inputs["Wv"], np.float32)
    in_common = {
        "wqt": np.ascontiguousarray(wq.T).astype(ml_dtypes.bfloat16),
        "wkt": np.ascontiguousarray(wk.T).astype(ml_dtypes.bfloat16),
        "wvt": np.ascontiguousarray(wv.T).astype(ml_dtypes.bfloat16),
        "bq": np.asarray(inputs["bq"], np.float32),
        "bk": np.asarray(inputs["bk"], np.float32),
        "bv16": np.asarray(inputs["bv"], np.float32).astype(ml_dtypes.bfloat16),
    }
    return [
        {"x": hst[i * BL : (i + 1) * BL], **in_common} for i in range(NCORES)
    ]


def kernel(**inputs) -> np.ndarray:
    in_maps = _make_in_maps(inputs)
    nc = _get_program()
    res = run_bass_kernel_spmd(nc, in_maps, list(range(NCORES)))
    return np.concatenate([res.results[i]["out"] for i in range(NCORES)], axis=0)

